# revision 6
# baseline (speedup 1.0000x reference)
"""Trainium2 Bass kernel for nn_ModelLSTM (36-step scalar-feedback LSTM).

Model (per reference):
    emb = relu(x @ W1.T + b1)                       # [B, 511], constant across steps
    x0 = -0.8; h0 = c0 = 0
    step t: inp = [emb, xin]                        # [B, 512]
            gates = inp @ Wih.T + bih + h @ Whh.T + bhh
            i,f,g,o = split(gates); c' = sig(f)*c + sig(i)*tanh(g); h' = sig(o)*tanh(c')
            y = h' @ W3.T + b3 ; xin' = y
    output ys = [36, B, 1]

Key algebraic restructuring (host-side, exact in fp64):
  * xin_t = y_{t-1} = h_t @ W3.T + b3, so the xin contribution to the gates
    folds into the recurrent weights:  Whh_eff = Whh + Wih[:,511:] @ W3,
    bias_eff = bih + bhh + b3 * Wih[:,511].
  * The emb contribution is constant across steps: A = Wih[:,:511] @ emb.T
    is precomputed once on device and replayed into PSUM each step via an
    identity matmul (PSUM cannot be pre-loaded by DMA/DVE for accumulation).
  So each step is ONE [2048+y,512]x[512,B] bf16 matmul + LSTM cell elementwise.

Sharding: pure data-parallel over batch (16384 -> 8 cores x 2048). Weights
replicated. No collectives.

Layout: everything transposed - batch is the free dim, gate/hidden index is
the partition dim. h'[k] tiles come out of the cell elementwise ops already
in the layout the next step's matmul consumes -> no transposes anywhere.
"""

import sys

for _p in ("/opt/trn_rl_repo",):
    if _p not in sys.path:
        sys.path.insert(0, _p)

import numpy as np
import ml_dtypes

BF16 = ml_dtypes.bfloat16

N_CORES = 8
B = 16384
BL = B // N_CORES  # 2048 batch per core
HID = 512
EMB = 511
STEPS = 36
NG = 4 * HID  # 2048 gate rows
M_TILES = 16  # gate row tiles of 128
K_TILES = 4  # contraction tiles of 128 over HID
NC_CH = BL // 512  # 4 free-dim chunks of 512


def _build_program(b3_val: float, n_steps: int = STEPS):
    import concourse.bass as bass
    import concourse.bacc as bacc
    import concourse.tile as tile
    from concourse import mybir

    fp32 = mybir.dt.float32
    bf16 = mybir.dt.bfloat16
    AF = mybir.ActivationFunctionType
    OP = mybir.AluOpType

    nc = bacc.Bacc(
        "TRN2",
        target_bir_lowering=False,
        debug=False,
        num_devices=N_CORES,
    )

    # ---- DRAM I/O (per-core shapes) ----
    xT_d = nc.dram_tensor("xT", [24, BL], bf16, kind="ExternalInput")
    w1t_d = nc.dram_tensor("w1t", [24, 512], bf16, kind="ExternalInput")
    wbig_d = nc.dram_tensor("wbig", [HID, NG + 128], bf16, kind="ExternalInput")
    we_d = nc.dram_tensor("we", [HID, NG], bf16, kind="ExternalInput")
    ident_d = nc.dram_tensor("ident", [128, 128], bf16, kind="ExternalInput")
    bias_st_d = nc.dram_tensor("bias_st", [128, M_TILES], fp32, kind="ExternalInput")
    bias_d0_d = nc.dram_tensor("bias_d0", [128, M_TILES], fp32, kind="ExternalInput")
    out_d = nc.dram_tensor("out", [STEPS, BL], fp32, kind="ExternalOutput")
    # scratch DRAM for the precomputed emb contribution A (streamed every step)
    a_d = nc.dram_tensor("a_scratch", [M_TILES, 128, BL], bf16, kind="Internal")

    with tile.TileContext(nc) as tc:
        with (
            tc.tile_pool(name="const", bufs=1) as constp,
            tc.tile_pool(name="state", bufs=1) as statep,
            tc.tile_pool(name="hpool", bufs=2) as hpool,
            tc.tile_pool(name="work", bufs=2) as workp,
            tc.tile_pool(name="astream", bufs=6) as apool,
            tc.tile_pool(name="psum", bufs=2, space=bass.MemorySpace.PSUM) as psump,
        ):
            # ---- load constants ----
            wbig_sb = []
            for k in range(K_TILES):
                wt = constp.tile([128, NG + 128], bf16, name=f"wbig{k}")
                nc.sync.dma_start(wt[:], wbig_d[k * 128 : (k + 1) * 128, :])
                wbig_sb.append(wt)
            ident_sb = constp.tile([128, 128], bf16, name="ident")
            nc.sync.dma_start(ident_sb[:], ident_d[:])
            bias_st = constp.tile([128, M_TILES], fp32, name="bias_st")
            nc.sync.dma_start(bias_st[:], bias_st_d[:])
            bias_d0 = constp.tile([128, M_TILES], fp32, name="bias_d0")
            nc.sync.dma_start(bias_d0[:], bias_d0_d[:])

            # ---- setup: emb = relu(x @ W1.T + b1) (transposed, bf16) ----
            # Setup tiles borrow main-pool tags (those slots are idle during
            # setup and setup tiles are dead before the main loop needs them).
            xT_sb = workp.tile([24, BL], bf16, tag="fc", name="xT_sb")
            nc.sync.dma_start(xT_sb[:], xT_d[:])
            w1t_sb = workp.tile([24, 512], bf16, tag="ig", name="w1t_sb")
            nc.sync.dma_start(w1t_sb[:], w1t_d[:])
            we_sb = []
            for k in range(K_TILES):
                wet = hpool.tile([128, NG], bf16, tag=f"h{k}", name=f"we{k}")
                nc.sync.dma_start(wet[:], we_d[k * 128 : (k + 1) * 128, :])
                we_sb.append(wet)

            embT = []
            for mj in range(4):
                eps = psump.tile([128, BL], fp32, tag="gps", name="eps")
                for ncn in range(NC_CH):
                    s = slice(ncn * 512, (ncn + 1) * 512)
                    nc.tensor.matmul(
                        eps[:, s],
                        w1t_sb[:, mj * 128 : (mj + 1) * 128],
                        xT_sb[:, s],
                        start=True,
                        stop=True,
                    )
                et = workp.tile([128, BL], bf16, tag=f"g{mj}", name=f"embT{mj}")
                nc.scalar.activation(et[:], eps[:], AF.Relu)
                embT.append(et)

            # ---- setup: A[m] = We @ embT + bias_steady -> DRAM (bf16) ----
            for m in range(M_TILES):
                aps = psump.tile([128, BL], fp32, tag="gps", name="aps")
                for k in range(K_TILES):
                    for ncn in range(NC_CH):
                        s = slice(ncn * 512, (ncn + 1) * 512)
                        nc.tensor.matmul(
                            aps[:, s],
                            we_sb[k][:, m * 128 : (m + 1) * 128],
                            embT[k][:, s],
                            start=(k == 0),
                            stop=(k == K_TILES - 1),
                        )
                ast = apool.tile([128, BL], bf16, tag="astream", name="astage")
                nc.scalar.activation(
                    ast[:], aps[:], AF.Identity, bias=bias_st[:, m : m + 1]
                )
                nc.sync.dma_start(a_d[m], ast[:])

            # ---- step 0: h0 = c0 = 0, xin = -0.8 ----
            # gates0 = A + (bias0 - bias_steady); c1 = sig(i)*tanh(g); h1 = sig(o)*tanh(c1)
            h_cur = [None] * K_TILES
            c_sb = [None] * K_TILES
            for hid in range(K_TILES):
                gact = {}
                for gt, func in ((0, AF.Sigmoid), (2, AF.Tanh), (3, AF.Sigmoid)):
                    m = gt * 4 + hid
                    ab = apool.tile([128, BL], bf16, tag="astream", name="a0buf")
                    nc.sync.dma_start(ab[:], a_d[m])
                    g = workp.tile([128, BL], fp32, tag=f"g{gt}", name=f"g{gt}_0")
                    nc.scalar.activation(g[:], ab[:], func, bias=bias_d0[:, m : m + 1])
                    gact[gt] = g
                ct = statep.tile([128, BL], fp32, name=f"c{hid}")
                nc.vector.tensor_tensor(ct[:], gact[0][:], gact[2][:], OP.mult)
                tt = workp.tile([128, BL], fp32, tag="fc", name="t0")
                nc.scalar.activation(tt[:], ct[:], AF.Tanh)
                ht = hpool.tile([128, BL], bf16, tag=f"h{hid}", name=f"h{hid}_0")
                nc.vector.tensor_tensor(ht[:], gact[3][:], tt[:], OP.mult)
                c_sb[hid] = ct
                h_cur[hid] = ht

            # ---- steps 1..35 ----
            GATE_FUNC = {0: AF.Sigmoid, 1: AF.Sigmoid, 2: AF.Tanh, 3: AF.Sigmoid}

            def gates_matmul(gps, m, h_in):
                """Accumulate psum <- A[m] (via identity) + Whh_eff[:,m-tile] @ h."""
                ab = apool.tile([128, BL], bf16, tag="astream", name="abuf")
                nc.sync.dma_start(ab[:], a_d[m])
                for ncn in range(NC_CH):
                    s = slice(ncn * 512, (ncn + 1) * 512)
                    nc.tensor.matmul(
                        gps[:, s], ident_sb[:], ab[:, s], start=True, stop=False
                    )
                for k in range(K_TILES):
                    for ncn in range(NC_CH):
                        s = slice(ncn * 512, (ncn + 1) * 512)
                        nc.tensor.matmul(
                            gps[:, s],
                            wbig_sb[k][:, m * 128 : (m + 1) * 128],
                            h_in[k][:, s],
                            start=False,
                            stop=(k == K_TILES - 1),
                        )

            def y_tile(t_out, h_in):
                """y = W3 @ h (+ b3) via the padded 17th weight tile -> out[t_out]."""
                gps = psump.tile([128, BL], fp32, tag="gps", name="yps")
                for k in range(K_TILES):
                    for ncn in range(NC_CH):
                        s = slice(ncn * 512, (ncn + 1) * 512)
                        nc.tensor.matmul(
                            gps[:, s],
                            wbig_sb[k][:, NG : NG + 128],
                            h_in[k][:, s],
                            start=(k == 0),
                            stop=(k == K_TILES - 1),
                        )
                yr = workp.tile([1, BL], fp32, tag="fc", name="yrow")
                nc.scalar.add(yr[:], gps[0:1, :], float(b3_val))
                nc.sync.dma_start(out_d[t_out : t_out + 1, :], yr[:])

            for t in range(1, n_steps):
                h_next = [None] * K_TILES
                for hid in range(K_TILES):
                    gact = {}
                    for gt in range(4):
                        m = gt * 4 + hid
                        gps = psump.tile([128, BL], fp32, tag="gps", name="gps")
                        gates_matmul(gps, m, h_cur)
                        g = workp.tile(
                            [128, BL], fp32, tag=f"g{gt}", name=f"g{gt}_{t}"
                        )
                        nc.scalar.activation(g[:], gps[:], GATE_FUNC[gt])
                        gact[gt] = g
                    # cell update for this hid tile
                    fc = workp.tile([128, BL], fp32, tag="fc", name="fc")
                    nc.vector.tensor_tensor(fc[:], gact[1][:], c_sb[hid][:], OP.mult)
                    ig = workp.tile([128, BL], fp32, tag="ig", name="ig")
                    nc.vector.tensor_tensor(ig[:], gact[0][:], gact[2][:], OP.mult)
                    nc.vector.tensor_tensor(c_sb[hid][:], fc[:], ig[:], OP.add)
                    tt = workp.tile([128, BL], fp32, tag="fc", name="tt")
                    nc.scalar.activation(tt[:], c_sb[hid][:], AF.Tanh)
                    ht = hpool.tile([128, BL], bf16, tag=f"h{hid}", name=f"h{hid}_{t}")
                    nc.vector.tensor_tensor(ht[:], gact[3][:], tt[:], OP.mult)
                    h_next[hid] = ht
                # y_{t-1} from h_cur (the h this step's matmuls consumed)
                y_tile(t - 1, h_cur)
                h_cur = h_next

            # final output y_{n-1} from the last h
            y_tile(n_steps - 1, h_cur)

    nc.compile()
    return nc


def _prepare_inputs(x, W1, b1, Wih, bih, Whh, bhh, W3, b3):
    """Host-side exact weight folding (fp64) + per-core sharding."""
    wih_col = Wih[:, 511:512].astype(np.float64)  # [2048,1]
    Whh_eff = Whh.astype(np.float64) + wih_col @ W3.astype(np.float64)  # [2048,512]
    bias_steady = (
        bih.astype(np.float64) + bhh.astype(np.float64) + wih_col[:, 0] * float(b3[0])
    )
    bias_d0 = (-0.8 * wih_col[:, 0] - wih_col[:, 0] * float(b3[0])).astype(np.float64)

    # lhsT layout [K=512, M=2048+128]: gates cols then y col (W3), zero padded
    wbig = np.zeros((HID, NG + 128), np.float64)
    wbig[:, :NG] = Whh_eff.T
    wbig[:, NG] = W3[0].astype(np.float64)

    we = np.zeros((HID, NG), np.float32)
    we[:EMB, :] = Wih[:, :EMB].T  # row 511 zero (emb row 511 is zero)

    w1t = np.zeros((24, 512), np.float32)
    w1t[:23, :EMB] = W1.T
    w1t[23, :EMB] = b1

    ident = np.eye(128, dtype=np.float32)

    bias_st_2d = bias_steady.reshape(M_TILES, 128).T.astype(np.float32)
    bias_d0_2d = bias_d0.reshape(M_TILES, 128).T.astype(np.float32)

    common = {
        "w1t": w1t.astype(BF16),
        "wbig": wbig.astype(np.float32).astype(BF16),
        "we": we.astype(BF16),
        "ident": ident.astype(BF16),
        "bias_st": np.ascontiguousarray(bias_st_2d),
        "bias_d0": np.ascontiguousarray(bias_d0_2d),
    }
    in_maps = []
    for c in range(N_CORES):
        xs = x[c * BL : (c + 1) * BL]  # [BL, 23]
        xT = np.ones((24, BL), np.float32)
        xT[:23, :] = xs.T
        m = dict(common)
        m["xT"] = np.ascontiguousarray(xT).astype(BF16)
        in_maps.append(m)
    return in_maps, float(b3[0])


def kernel(x, W1, b1, Wih, bih, Whh, bhh, W3, b3):
    from concourse.bass_utils import run_bass_kernel_spmd

    x = np.asarray(x, np.float32)
    in_maps, b3_val = _prepare_inputs(
        np.asarray(x, np.float32),
        np.asarray(W1, np.float32),
        np.asarray(b1, np.float32),
        np.asarray(Wih, np.float32),
        np.asarray(bih, np.float32),
        np.asarray(Whh, np.float32),
        np.asarray(bhh, np.float32),
        np.asarray(W3, np.float32),
        np.asarray(b3, np.float32),
    )
    nc = _build_program(b3_val)
    res = run_bass_kernel_spmd(nc, in_maps, list(range(N_CORES)))
    outs = [np.asarray(res.results[c]["out"]) for c in range(N_CORES)]  # [36, BL] each
    full = np.concatenate(outs, axis=1)  # [36, B]
    return full[:, :, None].astype(np.float32)  # [36, B, 1]


if __name__ == "__main__":
    rng = np.random.default_rng(0)
    ins = {
        "x": rng.standard_normal((B, 23), dtype=np.float32),
        "W1": rng.standard_normal((EMB, 23), dtype=np.float32) / np.sqrt(23),
        "b1": np.zeros(EMB, np.float32),
        "Wih": rng.standard_normal((NG, HID), dtype=np.float32) / np.sqrt(HID),
        "bih": np.zeros(NG, np.float32),
        "Whh": rng.standard_normal((NG, HID), dtype=np.float32) / np.sqrt(HID),
        "bhh": np.zeros(NG, np.float32),
        "W3": rng.standard_normal((1, HID), dtype=np.float32) / np.sqrt(HID),
        "b3": np.zeros(1, np.float32),
    }
    out = kernel(**ins)
    print("kernel output", out.shape, out.dtype, np.abs(out).max())
